# revision 1
# baseline (speedup 1.0000x reference)
"""Trainium2 Bass kernel for nn_DKOKernel (dense pairwise MLP + PSD head).

Math (per batch b, one NeuronCore per batch):
  hx[f,i] = sum_c wx[f,c] x[b,i,c];  hy[f,j] = sum_c wy[f,c] y[b,j,c]
  h1 = relu(bn1(hx_i + hy_j + b1))          (512)
  h2 = relu(bn2(W2 h1 + b2))                (256)
  h3 = relu(bn3(W3 h2 + b3))                (128)
  e  = W4 h3 + b4                           (64)
  out[b,i,j] = sum_k e[b,i,j,k] * (sum_l e[b,i,l,k])

BatchNorm affines are folded into weights/biases on the host.

Head algebra (avoids materializing e):
  q_i  = sum_j h3_ij
  v_i  = M q_i + ny*wc          (M = W4^T W4, wc = W4^T b4)
  c_i  = wc.q_i + ny*|b4|^2
  out[i,j] = h3_ij . v_i + c_i

Device layout: features on partitions, (i-block, j) pairs on the free dim,
C=4 i-rows per chunk => ap 512 (one PSUM bank, and the >=256 threshold).
Big matmuls run in bf16 (1 row/cycle and the PE sustains ~2GHz when fed
back-to-back; f32r measured ~1.7x slower per row on HW). h3/v are bf16 (the fp32r tiny-matmul ISA restrictions reject ap=1 fp32r).

The output is accumulated TRANSPOSED (outT[j,i]) in one held PSUM bank:
per chunk, a ones-row matmul adds c_i to its 4 columns and four ap=1
matmuls (stationary = h3 j-block, moving = v column) add h3.v; the host
transposes back. This removes the elementwise em-multiply and the
[1,512] column-sum/add of the previous design.

L1 split: fc group 0 is computed ON THE PE per chunk (x-part via a
selection-matrix matmul against hxT rows, y-part via a broadcast-AP
matmul over yT), evicted with relu+c1 by ACT; fc groups 1-3 are fused
add+relu tensor_scalar quarters on DVE/ACT (per-quarter engine string).
"""

import numpy as np
from contextlib import ExitStack

import concourse.bacc as bacc
import concourse.tile as tile
from concourse import mybir
from concourse.bass_utils import run_bass_kernel_spmd

F32 = mybir.dt.float32
F32R = mybir.dt.float32r
BF16 = mybir.dt.bfloat16
AF = mybir.ActivationFunctionType
ALU = mybir.AluOpType
AX = mybir.AxisListType

EPS = 1e-5
B = 8
N = 128          # nx == ny
F = 128          # input feature dim
D1, D2, D3, D4 = 512, 256, 128, 64
C = 4            # i-rows per chunk -> free dim C*N = 512
NCH = N // C

import os
# how many fc groups (from 0) computed on the PE; rest are DVE/ACT quarters
N_FC_PE = int(os.environ.get('N_FC_PE', '0'))
# engine per quarter for the non-PE fc groups: one char per (fc, ii),
# fc-major, starting at fc=N_FC_PE. V = DVE, A = ACT.
L1_ASSIGN = os.environ.get('L1_ASSIGN', 'VVVVVVVVVVVVVAAA')
# engines for the three PSUM evicts per chunk (h2 x2, h3): V/A per slot
EV_ASSIGN = os.environ.get('EV_ASSIGN', 'AAA')
# engine for the q row-sum reduce: V (DVE) or P (Pool)
Q_ENGINE = os.environ.get('Q_ENGINE', 'V')
# L2 contraction order (which h1 fc group each accumulation step reads)
KC_ORDER = [int(c) for c in os.environ.get('KC_ORDER', '3012')]
# where to emit the head v/c matmuls: 'mid' (between L2 halves) or 'late'
VC_SLOT = os.environ.get('VC_SLOT', 'late')
PAIR = int(os.environ.get('PAIR', '1'))
# engine for the tiny head evicts (v/c/po): A (ACT) or P (Pool)
HEADSMALL = os.environ.get('HEADSMALL', 'A')
# debug: dump an intermediate tile into the output instead of the result
DUMP = os.environ.get('DUMP', '')
HEAD_INLINE = os.environ.get('HEAD_INLINE', '0') == '1'
NO_CMM = os.environ.get('NO_CMM', '0') == '1'
HELD_POT = os.environ.get('HELD_POT', '1') == '1'


def build_module():
    nc = bacc.Bacc()

    xT = nc.declare_dram_parameter("xT", [F, N], F32, isOutput=False)
    yT = nc.declare_dram_parameter("yT", [F, N], F32, isOutput=False)
    wxT = nc.declare_dram_parameter("wxT", [F, D1], F32, isOutput=False)
    wyT = nc.declare_dram_parameter("wyT", [F, D1], F32, isOutput=False)
    w2T = nc.declare_dram_parameter("w2T", [128, 4, D2], F32, isOutput=False)
    w3T = nc.declare_dram_parameter("w3T", [128, 2, D3], F32, isOutput=False)
    Md = nc.declare_dram_parameter("M", [128, 128], F32, isOutput=False)
    Bd = nc.declare_dram_parameter("B", [128, 10], F32, isOutput=False)
    seld = nc.declare_dram_parameter("sel", [C, C * N], F32, isOutput=False)
    out_d = nc.declare_dram_parameter("out", [N, N], F32, isOutput=True)

    with tile.TileContext(nc) as tc:
        with ExitStack() as ctx:
            singles = ctx.enter_context(tc.tile_pool(name="singles", bufs=1))

            xT_f = singles.tile([F, N], F32)
            yT_f = singles.tile([F, N], F32)
            wxT_f = singles.tile([F, D1], F32)
            wyT_f = singles.tile([F, D1], F32)
            w2_f = singles.tile([128, 4, D2], F32)
            w3_f = singles.tile([128, 2, D3], F32)
            sel_f = singles.tile([C, C * N], F32)
            ones_f = singles.tile([1, 128], F32)

            xT_b = singles.tile([F, N], BF16)
            yT_b = singles.tile([F, N], BF16)
            wxT_b = singles.tile([F, D1], BF16)
            wyT_b = singles.tile([F, D1], BF16)
            w2_b = singles.tile([128, 4, D2], BF16)
            w3_b = singles.tile([128, 2, D3], BF16)
            sel_b = singles.tile([C, C * N], BF16)
            ones_b = singles.tile([1, 128], BF16)

            M_s = singles.tile([128, 128], F32)
            B_s = singles.tile([128, 10], F32)
            c1_s = B_s[:, 0:4]
            c2_s = B_s[:, 4:6]
            c3_s = B_s[:, 6:7]
            wc_s = B_s[:, 7:8]
            nywc_s = B_s[:, 8:9]
            c0_s = B_s[0:1, 9:10]

            q_all = singles.tile([128, N], F32)      # per-i row-sums of h3
            hxT_b = singles.tile([N, D1], BF16)      # [i, f] staging
            hxT_r = singles.tile([C, NCH, D1], BF16)  # [a, t, f] for PE L1
            hx_s = singles.tile([128, 4, N], F32)    # [f, i] for quarters
            hy_b = singles.tile([128, 4, N], BF16)   # [f, j] (+c1 folded)

            # per-block weight DMAs, dependency-first, spread over four
            # idle queues (each dispatch costs ~650ns serially per queue)
            nc.sync.dma_start(out=xT_f, in_=xT[:, :])
            nc.gpsimd.dma_start(out=wxT_f[:, 0:128], in_=wxT[:, 0:128])
            nc.scalar.dma_start(out=wyT_f[:, 0:128], in_=wyT[:, 0:128])
            nc.scalar.dma_start(out=yT_f, in_=yT[:, :])
            nc.sync.dma_start(out=wxT_f[:, 128:256], in_=wxT[:, 128:256])
            nc.gpsimd.dma_start(out=wxT_f[:, 256:384], in_=wxT[:, 256:384])
            nc.scalar.dma_start(out=wyT_f[:, 128:256], in_=wyT[:, 128:256])
            nc.gpsimd.dma_start(out=B_s, in_=Bd[:, :])
            nc.sync.dma_start(out=wxT_f[:, 384:512], in_=wxT[:, 384:512])
            nc.gpsimd.dma_start(out=w2_f[:, 3, :], in_=w2T[:, 3, :])
            nc.scalar.dma_start(out=wyT_f[:, 256:384], in_=wyT[:, 256:384])
            nc.sync.dma_start(out=wyT_f[:, 384:512], in_=wyT[:, 384:512])
            nc.sync.dma_start(out=w2_f[:, 0, :], in_=w2T[:, 0, :])
            nc.gpsimd.dma_start(out=w2_f[:, 1, :], in_=w2T[:, 1, :])
            nc.scalar.dma_start(out=w2_f[:, 2, :], in_=w2T[:, 2, :])
            nc.gpsimd.dma_start(out=w3_f, in_=w3T[:, :, :])
            nc.sync.dma_start(out=M_s, in_=Md[:, :])
            nc.vector.memset(ones_f, 1.0)
            if N_FC_PE > 0:
                nc.gpsimd.dma_start(out=sel_f, in_=seld[:, :])

            if N_FC_PE > 0:
                nc.vector.tensor_copy(out=xT_b, in_=xT_f)
                nc.vector.tensor_copy(out=yT_b, in_=yT_f)
                nc.vector.tensor_copy(out=wxT_b, in_=wxT_f)
                nc.vector.tensor_copy(out=wyT_b, in_=wyT_f)

            # ---- setup: hx (scalars), hy'' (+c1); fp32 mms (no cast dep) ----
            with tc.tile_pool(name="psum_setup", bufs=2, space="PSUM") as pp:
                for fc in range(4):
                    ph = pp.tile([128, N], F32, tag="ps", name="ph")
                    nc.tensor.matmul(
                        ph, lhsT=wxT_f[:, fc * 128:(fc + 1) * 128],
                        rhs=xT_f, start=True, stop=True)
                    nc.scalar.activation(hx_s[:, fc, :], ph, AF.Copy)
                    py_ = pp.tile([128, N], F32, tag="ps2", name="py_")
                    nc.tensor.matmul(
                        py_, lhsT=wyT_f[:, fc * 128:(fc + 1) * 128],
                        rhs=yT_f, start=True, stop=True)
                    nc.scalar.activation(hy_b[:, fc, :], py_, AF.Identity,
                                         bias=c1_s[:, fc:fc + 1])
                if N_FC_PE > 0:
                    # hxT[i, f] then regroup so chunk t's rows sit at
                    # partitions 0-3 (DMA partition steps must be 1, so
                    # bounce through DRAM)
                    pxt = pp.tile([N, D1], F32, tag="ps", name="pxt")
                    nc.tensor.matmul(pxt, lhsT=xT_b, rhs=wxT_b,
                                     start=True, stop=True)
                    nc.scalar.activation(hxT_b, pxt, AF.Copy)
                    hxTr_d = nc.dram_tensor("hxTr_scratch", [C, NCH, D1],
                                            BF16, kind="Internal")
                    nc.sync.dma_start(
                        out=hxTr_d.rearrange("a t f -> t a f"),
                        in_=hxT_b.rearrange("(t a) f -> (t a) f", a=C))
                    nc.sync.dma_start(out=hxT_r, in_=hxTr_d[:, :, :])

            # weight casts after the setup matmuls so they don't gate them
            for kc in (3, 0, 1, 2):
                nc.vector.tensor_copy(out=w2_b[:, kc, :], in_=w2_f[:, kc, :])
            nc.vector.tensor_copy(out=w3_b, in_=w3_f)
            nc.vector.tensor_copy(out=ones_b, in_=ones_f)
            if N_FC_PE > 0:
                nc.vector.tensor_copy(out=sel_b, in_=sel_f)

            work = ctx.enter_context(tc.tile_pool(name="work", bufs=3))
            if N_FC_PE > 0:
                psum1 = ctx.enter_context(tc.tile_pool(name="psum1", bufs=2,
                                                       space="PSUM"))
            psum2 = ctx.enter_context(tc.tile_pool(name="psum2", bufs=2,
                                                   space="PSUM"))
            psum3 = ctx.enter_context(tc.tile_pool(
                name="psum3", bufs=(1 if N_FC_PE > 0 else 2), space="PSUM"))
            psumh = ctx.enter_context(tc.tile_pool(name="psumh", bufs=1,
                                                   space="PSUM"))
            if HELD_POT:
                psumo = ctx.enter_context(
                    tc.tile_pool(name="psumo", bufs=1, space="PSUM"))
                poT = psumo.tile([N, N], F32)  # held outT[j, i] accumulator
            outT_s = singles.tile([N, N], F32)

            def emit_L1(t):
                h1 = work.tile([128, 4, C * N], BF16, tag="h1", name="h1")
                for fc in range(N_FC_PE):
                    p1 = psum1.tile([128, C * N], F32, tag="p1", name="p1")
                    # x part: hxT rows 4t..4t+4 against the selection matrix
                    nc.tensor.matmul(
                        p1, lhsT=hxT_r[:, t, fc * 128:(fc + 1) * 128],
                        rhs=sel_b, start=True, stop=False)
                    # y part: wy block against yT broadcast along ii
                    nc.tensor.matmul(
                        p1, lhsT=wyT_b[:, fc * 128:(fc + 1) * 128],
                        rhs=yT_b.unsqueeze(1).broadcast_to([F, C, N]),
                        start=False, stop=True)
                    nc.scalar.activation(h1[:, fc, :], p1, AF.Relu,
                                         bias=c1_s[:, fc:fc + 1])
                for fc in range(N_FC_PE, 4):
                    modes = L1_ASSIGN[(fc - N_FC_PE) * C:
                                      (fc - N_FC_PE + 1) * C]
                    for ii in range(C):
                        sl = slice(ii * N, (ii + 1) * N)
                        xc = hx_s[:, fc, C * t + ii:C * t + ii + 1]
                        if modes[ii] == "V":
                            nc.vector.tensor_scalar(
                                out=h1[:, fc, sl], in0=hy_b[:, fc, :],
                                scalar1=xc, scalar2=0.0,
                                op0=ALU.add, op1=ALU.max)
                        else:
                            nc.scalar.activation(
                                h1[:, fc, sl], hy_b[:, fc, :],
                                AF.Relu, bias=xc)
                return h1

            def emit_head_vc(st):
                tp, h3s, _ = st
                W = C * len(h3s)
                ps_v = psumh.tile([128, 2 * C], F32, tag="hv", name="ps_v")
                nc.tensor.matmul(ps_v[:, 0:W], lhsT=M_s,
                                 rhs=q_all[:, C * tp:C * tp + W],
                                 start=True, stop=True)
                v_sb = work.tile([128, 2 * C], BF16, tag="v", name="v_sb")
                ps_c = psumh.tile([1, 2 * C], F32, tag="hc", name="ps_c")
                nc.tensor.matmul(ps_c[:, 0:W], lhsT=wc_s,
                                 rhs=q_all[:, C * tp:C * tp + W],
                                 start=True, stop=True)
                c_sb = work.tile([1, 2 * C], BF16, tag="c", name="c_sb")
                if HEADSMALL == 'V':
                    nc.vector.tensor_scalar(
                        out=v_sb[:, 0:W], in0=ps_v[:, 0:W], scalar1=nywc_s,
                        scalar2=None, op0=ALU.add)
                    nc.vector.tensor_scalar(
                        out=c_sb[:, 0:W], in0=ps_c[:, 0:W], scalar1=c0_s,
                        scalar2=None, op0=ALU.add)
                else:
                    nc.scalar.activation(v_sb[:, 0:W], ps_v[:, 0:W],
                                         AF.Identity, bias=nywc_s)
                    nc.scalar.activation(c_sb[:, 0:W], ps_c[:, 0:W],
                                         AF.Identity, bias=c0_s)
                return (v_sb, c_sb)

            def emit_head_out(st, v_sb, c_sb):
                # outT[j, 4tp+a] = c[a] (ones-row matmul) + h3(:,a-blk).v_a
                tp, h3s, _ = st
                W = C * len(h3s)
                po_c = (poT[:, C * tp:C * tp + W] if HELD_POT else
                        psumh.tile([N, 2 * C], F32, tag="poc",
                                   name="po_c")[:, 0:W])
                nc.tensor.matmul(po_c, lhsT=ones_b, rhs=c_sb[:, 0:W],
                                 start=True, stop=False)
                for k, h3_t in enumerate(h3s):
                    for a in range(C):
                        nc.tensor.matmul(
                            po_c[:, k * C + a:k * C + a + 1],
                            lhsT=h3_t[:, a * N:(a + 1) * N],
                            rhs=v_sb[:, k * C + a:k * C + a + 1],
                            start=False,
                            stop=(k == len(h3s) - 1 and a == C - 1))
                if not HELD_POT:
                    nc.scalar.activation(outT_s[:, C * tp:C * tp + W], po_c,
                                         AF.Copy)

            head_st = None
            pair = None  # (t0, [h3...], q2) for the pair being accumulated
            h1_cur = emit_L1(0)
            for t in range(NCH):
                h1_next = emit_L1(t + 1) if t + 1 < NCH else None

                # ---- L2 (head v/c for the previous pair in mid slot) ----
                h2 = work.tile([128, 2, C * N], BF16, tag="h2", name="h2")
                vc = None
                for mc in range(2):
                    p2 = psum2.tile([128, C * N], F32, tag="p2", name="p2")
                    for i_kc, kc in enumerate(KC_ORDER):
                        nc.tensor.matmul(
                            p2, lhsT=w2_b[:, kc, mc * 128:(mc + 1) * 128],
                            rhs=h1_cur[:, kc, :],
                            start=(i_kc == 0), stop=(i_kc == 3))
                    if EV_ASSIGN[mc] == "A":
                        nc.scalar.activation(h2[:, mc, :], p2, AF.Relu,
                                             bias=c2_s[:, mc:mc + 1])
                    else:
                        nc.vector.tensor_scalar(
                            out=h2[:, mc, :], in0=p2,
                            scalar1=c2_s[:, mc:mc + 1], scalar2=0.0,
                            op0=ALU.add, op1=ALU.max)
                    if VC_SLOT == 'mid' and mc == 0 and head_st is not None:
                        vc = emit_head_vc(head_st)

                # ---- L3 ----
                h3 = work.tile([128, C * N], BF16, tag="h3", name="h3")
                p3 = psum3.tile([128, C * N], F32, tag="p3", name="p3")
                for kc in range(2):
                    nc.tensor.matmul(
                        p3, lhsT=w3_b[:, kc, :], rhs=h2[:, kc, :],
                        start=(kc == 0), stop=(kc == 1))
                if pair is None:
                    pair = (t, [], None)
                if EV_ASSIGN[2] == "A":
                    nc.scalar.activation(h3, p3, AF.Relu, bias=c3_s[:, 0:1])
                else:
                    nc.vector.tensor_scalar(
                        out=h3, in0=p3, scalar1=c3_s[:, 0:1],
                        scalar2=0.0, op0=ALU.add, op1=ALU.max)
                nc.vector.tensor_reduce(
                    out=q_all[:, C * t:C * t + C],
                    in_=h3.rearrange("p (a b) -> p a b", a=C),
                    axis=AX.X, op=ALU.add)
                pair[1].append(h3)

                if DUMP.startswith('hxr') and t == 0:
                    td = int(DUMP.split(':')[1])
                    nc.vector.tensor_copy(
                        out=outT_s[0:C, :],
                        in_=hxT_r[:, td, 0:N])
                elif DUMP and not DUMP.startswith('po'):
                    parts = DUMP.split(':')
                    dt_ = parts[0]
                    idx = int(parts[1]) if len(parts) > 1 else 0
                    td = int(parts[2]) if len(parts) > 2 else 0
                    if t == td:
                        src = {'h1': h1_cur[:, idx, 0:N],
                               'h2': h2[:, idx, 0:N],
                               'h3': h3[:, idx * N:(idx + 1) * N]}[dt_]
                        nc.vector.tensor_copy(out=outT_s[:, 0:N], in_=src)

                if head_st is not None:
                    if VC_SLOT != 'mid':
                        vc = emit_head_vc(head_st)
                    emit_head_out(head_st, *vc)
                    head_st = None
                if HELD_POT and not DUMP and t == NCH // 2 + 1:
                    # first half of poT is complete (head lags one pair)
                    nc.scalar.activation(outT_s[:, 0:N // 2],
                                         poT[:, 0:N // 2], AF.Copy)
                    nc.sync.dma_start(out=out_d[:, 0:N // 2],
                                      in_=outT_s[:, 0:N // 2])
                if not DUMP and len(pair[1]) == PAIR:
                    head_st = pair
                    pair = None
                h1_cur = h1_next

            if head_st is not None:
                vc = emit_head_vc(head_st)
                emit_head_out(head_st, *vc)
                head_st = None
            if HELD_POT:
                nc.scalar.activation(outT_s[:, N // 2:], poT[:, N // 2:],
                                     AF.Copy)

            nc.sync.dma_start(out=out_d[:, N // 2:],
                              in_=outT_s[:, N // 2:])
    nc.finalize()
    return nc


_NC_CACHE = None


def _get_nc():
    global _NC_CACHE
    if _NC_CACHE is None:
        _NC_CACHE = build_module()
    return _NC_CACHE


def host_prep(inputs):
    """Fold the BatchNorm affines into weights/biases; pre-transpose
    everything into the device layouts. Returns the per-core input maps."""
    f32 = np.float32
    x = np.asarray(inputs["x"], f32)
    y = np.asarray(inputs["y"], f32)
    w1, b1 = np.asarray(inputs["w1"], f32), np.asarray(inputs["b1"], f32)
    w2, b2 = np.asarray(inputs["w2"], f32), np.asarray(inputs["b2"], f32)
    w3, b3 = np.asarray(inputs["w3"], f32), np.asarray(inputs["b3"], f32)
    w4, b4 = np.asarray(inputs["w4"], f32), np.asarray(inputs["b4"], f32)

    k1 = inputs["g1"] / np.sqrt(inputs["v1"] + EPS)
    c1 = k1 * (b1 - inputs["m1"]) + inputs["be1"]
    k2 = inputs["g2"] / np.sqrt(inputs["v2"] + EPS)
    c2 = k2 * (b2 - inputs["m2"]) + inputs["be2"]
    k3 = inputs["g3"] / np.sqrt(inputs["v3"] + EPS)
    c3 = k3 * (b3 - inputs["m3"]) + inputs["be3"]

    wx = w1[:, :F] * k1[:, None]          # (512, 128)
    wy = w1[:, F:] * k1[:, None]
    w2f = w2 * k2[:, None]                # (256, 512)
    w3f = w3 * k3[:, None]                # (128, 256)

    sel = np.zeros((C, C * N), f32)
    for a in range(C):
        sel[a, a * N:(a + 1) * N] = 1.0

    Bm = np.zeros((128, 10), f32)
    Bm[:, 0:4] = c1.reshape(4, 128).T
    Bm[:, 4:6] = c2.reshape(2, 128).T
    Bm[:, 6] = c3
    Bm[:, 7] = w4.T @ b4
    Bm[:, 8] = N * (w4.T @ b4)
    Bm[0, 9] = N * float(b4 @ b4)

    shared = {
        "wxT": np.ascontiguousarray(wx.T, f32),                    # (128, 512)
        "wyT": np.ascontiguousarray(wy.T, f32),
        "w2T": np.ascontiguousarray(
            w2f.T.reshape(4, 128, D2).transpose(1, 0, 2), f32),
        "w3T": np.ascontiguousarray(
            w3f.T.reshape(2, 128, D3).transpose(1, 0, 2), f32),
        "M": np.ascontiguousarray(w4.T @ w4, f32),                 # (128, 128)
        "B": Bm,
        "sel": sel,
    }
    in_maps = []
    for b in range(B):
        m = dict(shared)
        m["xT"] = np.ascontiguousarray(x[b].T, f32)
        m["yT"] = np.ascontiguousarray(y[b].T, f32)
        in_maps.append(m)
    return in_maps


def gather(res, inputs):
    """Device returns outT[j,i]; transpose back per batch."""
    outs = [res.results[b]["out"].T for b in range(B)]
    return np.ascontiguousarray(np.stack(outs, axis=0), np.float32)


def kernel(**inputs):
    nc = _get_nc()
    in_maps = host_prep(inputs)
    res = run_bass_kernel_spmd(nc, in_maps, list(range(B)))
    return gather(res, inputs)

